# revision 75
# baseline (speedup 1.0000x reference)
"""Trainium2 Bass kernel for nn_AttentionSimilarity.

Contract: kernel(**inputs) takes the FULL unsharded inputs (numpy) and
returns the FULL [64, 64] similarity matrix, distributing work across 8
NeuronCores internally.

Structure:
  prog1 (projections, sharded by batch): each core projects its 8
    a-batches and 8 b-batches through the three two-layer MLPs,
    emitting qaT/kaT/vaT/qbT/kbT/vbT chunks in [inner, (batch, n)]
    layout. Host gathers the a-side to full tensors.
  prog2 (attention, sharded by p = b-side batch): each core computes
    both attention paths for its 8 p's against all 64 q's, the cosine
    numerators/denominators via selector matmuls on the PE, and the
    per-(p,q) sums over n. Host assembles the [64, 64] output.

Math notes:
  - softmax feeds only cosine similarity, which is scale-invariant in
    the aligned vector, so the softmax max-shift and denominator cancel:
    softmax reduces to exp(scores/8).
  - the x-side cosine norm is folded on the host (vhat = v / max(|v|, eps)).
  - 1/max(|y|, eps) and the dot with vhat are applied on the host from
    the streamed-out aligned values.

Performance notes (vs the first working version):
  - prog1 W1 layer and both programs' score matmuls run in fp8e4 with
    MatmulPerfMode.DoubleRow (2 contraction rows per PE partition, 0.5
    cycles/output column): weights/features/q/k are DR-packed on the
    host ([K/2, 2, M] with k = (K/2)*s + p; lhsT m-blocks padded to
    MP=112 so the DR pair-stride stays 16-byte aligned).
  - the entire cosine stage (dot, squared-norm, rsqrt, mean over n)
    is computed on the HOST: the aligned values (As, bf16) stream out
    over the otherwise-idle DMA engines, deleting the M/SQ multiplies,
    all selector-reduce matmuls, the P1/P2 PSUM accumulators (freeing
    banks for aligned double-buffering), and the device epilogues.
    The device does projections, scores, softmax-exp and the aligned
    matmuls -- all of the O(B^2 N^2) compute.
  - warmup/tail: weight DMAs are split/consolidated so the first matmul
    starts as early as possible; prog1's W2 PSUM/copy/DMA pipeline is
    chunked per bank so stores drain during compute; path2 score tiles
    are 1536 columns (3 PSUM banks) to amortize the fixed per-
    instruction ACT access latency on the softmax exp, which is the
    saturated engine (~98% busy) in the final balance.
  - measured rel err vs fp32 reference: ~1.7e-3.

Dead end (measured): packing score tiles to 128 partitions by mixing
(q, m) across rows would cut exp columns 100/128, but the follow-up
aligned matmuls need operand slices at arbitrary partition offsets and
the PE requires base partition 0/32/64 (bass matmul assert); since
100 is not a multiple of 32, per-q slices of a packed layout are
unaddressable. The [m<=100, cols] score layout is forced.
"""

import os
import sys

sys.path.insert(0, "/opt/trn_rl_repo")
os.environ.setdefault("NEURON_RT_RESET_CORES", "1")

import numpy as np
import ml_dtypes  # noqa: F401  (bf16 host arrays)

import bass_rust
import concourse.bass as bass
import concourse.mybir as mybir
import concourse.tile as tile
from concourse.bass_utils import run_bass_kernel_spmd

F32 = mybir.dt.float32
F32R = mybir.dt.float32r
BF16 = mybir.dt.bfloat16
F16 = mybir.dt.float16
F8E4 = mybir.dt.float8e4
AF = mybir.ActivationFunctionType
DR = mybir.MatmulPerfMode.DoubleRow

B = 64          # batches per side
C = 512         # channels
N = 100         # H*W tokens per batch
INNER = 64      # projected dim
CORES = 8
PB = B // CORES  # batches per core (8)
BN = PB * N      # 800: (batch, n) columns per core chunk
EPS = 1e-8
KT1 = C // 128   # prog1 contraction tiles (4)
MP = 112         # fp8-DR padded m stride (112 % 16 == 0, >= N)

E1_BUFS = int(os.environ.get("K_E1_BUFS", "5"))
SEL_LAG = int(os.environ.get("K_SEL_LAG", "4"))
POOL_MOD1 = int(os.environ.get("K_POOL_MOD1", os.environ.get("K_POOL_MOD", "3")))
POOL_MOD2 = int(os.environ.get("K_POOL_MOD2", os.environ.get("K_POOL_MOD", "2")))
SEL_LAG2 = int(os.environ.get("K_SEL_LAG2", "4"))
M2_BUFS = int(os.environ.get("K_M2_BUFS", "8"))
MPOOL_MOD = int(os.environ.get("K_MPOOL_MOD", "0"))  # 0=never, k=every kth M on pool
M_BUFS = int(os.environ.get("K_M_BUFS", "8"))
E2_BUFS = int(os.environ.get("K_E2_BUFS", "3"))
S1_BUFS = int(os.environ.get("K_S1_BUFS", "2"))
A1_BUFS = int(os.environ.get("K_A1_BUFS", "1"))

_waitsplit_ctr = [0]


def _split_multi_waits(nc, max_waits=1):
    """This container's walrus build accepts at most ONE sync wait per
    instruction; Tile attaches several. Move extras onto preceding
    same-engine NoOps (engines are in-order, so semantics hold)."""
    n_split = 0
    for f in nc.m.functions:
        for blk in f.blocks:
            insts = list(blk.instructions)
            new_list = []
            changed = False
            for inst in insts:
                si = inst.sync_info
                waits = list(si.on_wait) if (si is not None and si.on_wait) else []
                if len(waits) > max_waits:
                    for w in waits[:-max_waits]:
                        _waitsplit_ctr[0] += 1
                        nop = mybir.InstNoOp(
                            name=f"I-waitsplit-{_waitsplit_ctr[0]}",
                            engine=inst.engine,
                            ins=[],
                            outs=[],
                            sync_info=bass_rust.SyncInfo(on_wait=[w], on_update=[]),
                        )
                        nc.register_instruction(nop, overwrite=True)
                        new_list.append(nop)
                        n_split += 1
                    si.on_wait = waits[-max_waits:]
                    inst.sync_info = si
                    changed = True
                new_list.append(inst)
            if changed:
                blk.instructions = new_list
    return n_split


# ---------------------------------------------------------------- prog1

def build_prog1():
    """Projection program. Per-core inputs:
      fa8dr, fb8dr: [KT*64, 2*BN] f8e4 DoubleRow-packed features
        (row kt*64+p, col (s, (b n)) holds feat[c = 128*kt + 64*s + p])
      wq1dr/...: [KT*64, 2*C] f8e4 DR weights (col (s, c_out))
      wq2/...: [C, INNER] bf16
    Outputs: qaT8/kaT8/vaT8/qbT8/kbT8/vbT8: [INNER, BN]  ([i, (b n)])
    """
    nc = bass.Bass("TRN2", target_bir_lowering=False, debug=False,
                   num_devices=CORES)
    fa8 = nc.dram_tensor("fa8dr", [KT1 * 64, 2 * BN], F8E4,
                         kind="ExternalInput").ap()
    fb8 = nc.dram_tensor("fb8dr", [KT1 * 64, 2 * BN], F8E4,
                         kind="ExternalInput").ap()
    w1 = {p: nc.dram_tensor(f"w{p}1dr", [KT1 * 64, 2 * C], F8E4,
                            kind="ExternalInput").ap()
          for p in "qkv"}
    w2 = {p: nc.dram_tensor(f"w{p}2", [C, INNER], BF16, kind="ExternalInput").ap()
          for p in "qkv"}
    outs = {(s, p): nc.dram_tensor(f"{p}{s}T8", [INNER, BN], F16,
                                   kind="ExternalOutput").ap()
            for s in "ab" for p in "qkv"}

    KT = KT1  # 4 contraction tiles of 128 (64 partitions x 2 DR)
    CT = C // 128  # 4 c_out tiles
    CH = [(0, 512), (512, BN)]  # psum-bank-aligned column chunks of BN

    with tile.TileContext(nc) as tc:
        with (
            tc.tile_pool(name="wpool", bufs=1) as wpool,
            tc.tile_pool(name="fpool", bufs=int(os.environ.get("K_F_BUFS", "2"))) as fpool,
            tc.tile_pool(name="hpool", bufs=int(os.environ.get("K_H_BUFS", "4"))) as hpool,
            tc.tile_pool(name="opool", bufs=2) as opool,
            tc.tile_pool(name="psH", bufs=int(os.environ.get("K_PSH_BUFS", "3")), space="PSUM") as psHp,
            tc.tile_pool(name="psO", bufs=int(os.environ.get("K_PSO_BUFS", "1")), space="PSUM") as psOp,
        ):
            w1sb, w2sb = {}, {}

            def load_w1(p):
                wt = wpool.tile([64, KT * 2 * C], F8E4, tag=f"w1{p}",
                                name=f"w1{p}sb")
                wv = wt[:].rearrange("p (kt x) -> p kt x", kt=KT)
                dv = w1[p].rearrange("(kt p) x -> p kt x", p=64)
                nc.sync.dma_start(wv[:, 0:2], dv[:, 0:2])
                nc.sync.dma_start(wv[:, 2:KT], dv[:, 2:KT])
                w1sb[p] = wt

            def load_w(p):
                load_w1(p)
                w2sb[p] = wpool.tile([128, KT * INNER], BF16, tag=f"w2{p}",
                                     name=f"w2{p}sb")
                nc.sync.dma_start(
                    w2sb[p][:].rearrange("p (kt i) -> p kt i", kt=KT),
                    w2[p].rearrange("(kt p) i -> p kt i", p=128))

            for s, feat in (("a", fa8), ("b", fb8)):
                fts = []
                for kt in range(KT):
                    if s == "a" and kt == 0:
                        load_w1("q")
                    ft = fpool.tile([64, 2 * BN], F8E4, tag=f"f{kt}")
                    nc.sync.dma_start(ft[:], feat[64 * kt:64 * (kt + 1), :])
                    fts.append(ft)
                if s == "a":
                    w2sb["q"] = wpool.tile([128, KT * INNER], BF16, tag="w2q",
                                           name="w2qsb")
                    nc.sync.dma_start(
                        w2sb["q"][:].rearrange("p (kt i) -> p kt i", kt=KT),
                        w2["q"].rearrange("(kt p) i -> p kt i", p=128))
                    load_w("k")
                    load_w("v")
                for p in "qkv":
                    hts = []
                    for t in range(CT):
                        psH = psHp.tile([128, 1024], F32, tag="psH")
                        for lo, hi in CH:
                            for kt in range(KT):
                                nc.tensor.matmul(
                                    psH[:, lo:hi],
                                    w1sb[p][:].rearrange(
                                        "p (kt two c) -> p kt two c",
                                        kt=KT, two=2)[
                                        :, kt, :, 128 * t:128 * t + 128],
                                    fts[kt][:].rearrange(
                                        "p (two n) -> p two n", two=2)[
                                        :, :, lo:hi],
                                    start=(kt == 0), stop=(kt == KT - 1),
                                    perf_mode=mybir.MatmulPerfMode.DoubleRow)
                        ht = hpool.tile([128, BN], BF16, tag=f"h{t}")
                        if t % 2 == 0:
                            nc.scalar.activation(ht[:], psH[:, 0:BN], AF.Relu)
                        else:
                            nc.vector.tensor_scalar_max(ht[:], psH[:, 0:BN],
                                                        0.0)
                        hts.append(ht)
                    psOs = [psOp.tile([INNER, 512], F32, tag="psOa",
                                      name="psOa"),
                            psOp.tile([INNER, 512], F32, tag="psOb",
                                      name="psOb")]
                    ot = opool.tile([INNER, BN], F16, tag="out")
                    for ci, (lo, hi) in enumerate(CH):
                        for kt in range(KT):
                            nc.tensor.matmul(
                                psOs[ci][:, 0:hi - lo],
                                w2sb[p][:, INNER * kt:INNER * (kt + 1)],
                                hts[kt][:, lo:hi],
                                start=(kt == 0), stop=(kt == KT - 1))
                        nc.scalar.copy(ot[:, lo:hi], psOs[ci][:, 0:hi - lo])
                        nc.sync.dma_start(outs[(s, p)][:, lo:hi],
                                          ot[:, lo:hi])

    _split_multi_waits(nc)
    return nc


# ---------------------------------------------------------------- prog2

def build_prog2():
    """Attention program, sharded over p (this core's 8 b-batches).

    Inputs (f32r unless noted):
      kaT, qaT      [INNER, B*N]   a-side K^T / Q^T, i on partitions
      qbT, kbT      [INNER, BN]    this core's b-side chunks
      vaL, vaR      [N, 32*128]    per q-pair j: [va[2j] | 0], [0 | va[2j+1]]
      vbL, vbR      [N, 8*128]     per p: [vb[p] | 0], [0 | vb[p]]
      vhat_bT2 f32  [128, BN]      v̂b^T twice (rows 0:64 and 64:128)
      vhat_aT2 f32  [128, B*N//2]  v̂a^T in chunk-pair layout
      master1, master8 [128, 320]  reduce-selector constants
    Outputs (f32):
      out1 [64, PB]   path1 per-(q, p) sums over n of cos1
      out2 [128, 4]   path2 sums; row r: chunk=r//8, p=r%8; q=4*(r//8)+col
    """
    nc = bass.Bass("TRN2", target_bir_lowering=False, debug=False,
                   num_devices=CORES)
    din = {}
    for name, shape, dt in [
        ("kaTdr", [32, 2 * B * MP], F8E4), ("qaTdr", [32, 2 * B * N], F8E4),
        ("qbTdr", [32, 2 * BN], F8E4), ("kbTdr", [32, 2 * PB * MP], F8E4),
        ("vaL", [N, (B // 2) * 128], F16), ("vaR", [N, (B // 2) * 128], F16),
        ("vbL", [N, PB * 128], F16), ("vbR", [N, PB * 128], F16),
    ]:
        din[name] = nc.dram_tensor(name, shape, dt, kind="ExternalInput").ap()
    as1o = nc.dram_tensor("as1o", [128, 32 * BN], BF16,
                          kind="ExternalOutput").ap()
    as2o = nc.dram_tensor("as2o", [128, 32 * 800], BF16,
                          kind="ExternalOutput").ap()

    CH1 = [(0, 512), (512, BN)]          # path1 column chunks of (p n)
    E2W = 1536                           # path2 score-chunk width (3 banks)
    NQ2 = (B * N) // E2W                 # 4 full chunks + 256 remainder
    E2CH = [(E2W * j, min(E2W * (j + 1), B * N)) for j in range(NQ2 + 1)]

    with tile.TileContext(nc) as tc:
        from contextlib import ExitStack
        with ExitStack() as ctx:
            inp = ctx.enter_context(tc.tile_pool(name="inp", bufs=1))
            sb = {}

            def load(name, ap=None, cols=None, cname=None):
                ap = din[name] if ap is None else ap
                if cols is not None:
                    ap = ap[:, cols[0]:cols[1]]
                cname = cname or name
                t = inp.tile(list(ap.shape), ap.dtype, tag=cname,
                             name=f"sb_{cname}")
                nc.sync.dma_start(t[:], ap[:])
                sb[cname] = t

            # path1-critical tensors first (chunked so compute starts early)
            # kaTdr in 4 q-chunks of 16 (both DR halves per chunk)
            ka_t = inp.tile([32, 2 * B * MP], F8E4, tag="kaTdr",
                            name="sb_kaTdr")
            sb["kaTdr"] = ka_t
            ka3d = din["kaTdr"].rearrange("p (two q m) -> p two q m",
                                          two=2, q=B)
            ka3s = ka_t[:].rearrange("p (two q m) -> p two q m", two=2, q=B)
            nc.sync.dma_start(ka3s[:, :, 0:16, :], ka3d[:, :, 0:16, :])
            load("qbTdr")
            load("vaL", cols=(0, 512), cname="vaL0")
            load("vaR", cols=(0, 512), cname="vaR0")
            for c in range(1, 4):
                nc.sync.dma_start(ka3s[:, :, 16 * c:16 * (c + 1), :],
                                  ka3d[:, :, 16 * c:16 * (c + 1), :])
            # vaL/vaR tail in 2 big chunks each, sliced into per-512 views
            for nm in ("vaL", "vaR"):
                big = inp.tile([N, 3584], F16, tag=f"{nm}big",
                               name=f"sb_{nm}big")
                nc.sync.dma_start(big[:, 0:1536], din[nm][:, 512:2048])
                nc.sync.dma_start(big[:, 1536:3584], din[nm][:, 2048:4096])
                for c in range(1, 8):
                    sb[f"{nm}{c}"] = None
                sb[f"{nm}big"] = big
            # qaTdr in 2 n-chunks (both DR halves per chunk)
            qa_t = inp.tile([32, 2 * B * N], F8E4, tag="qaTdr",
                            name="sb_qaTdr")
            sb["qaTdr"] = qa_t
            qa3d = din["qaTdr"].rearrange("p (two n) -> p two n", two=2)
            qa3s = qa_t[:].rearrange("p (two n) -> p two n", two=2)
            nc.sync.dma_start(qa3s[:, :, 0:3200], qa3d[:, :, 0:3200])
            nc.sync.dma_start(qa3s[:, :, 3200:6400], qa3d[:, :, 3200:6400])
            for name in ("kbTdr", "vbL", "vbR"):
                load(name)

            epool = ctx.enter_context(tc.tile_pool(name="epool", bufs=E1_BUFS))
            mpool = ctx.enter_context(tc.tile_pool(name="mpool", bufs=M_BUFS))
            fin = ctx.enter_context(tc.tile_pool(name="fin", bufs=1))

            # ---------------- path 1: per q-pair over this core's (p n) ----
            with (
                tc.tile_pool(name="ps_s1", bufs=S1_BUFS, space="PSUM") as ps_s1,
                tc.tile_pool(name="ps_a1", bufs=int(os.environ.get("K_A1_BUFS","2")), space="PSUM") as ps_a1,
            ):
                ka3 = sb["kaTdr"][:].rearrange("p (two q m) -> p two q m",
                                               two=2, q=B)
                qb3 = sb["qbTdr"][:].rearrange("p (two n) -> p two n", two=2)
                for j in range(B // 2):
                    q0, q1 = 2 * j, 2 * j + 1
                    Ss = [ps_s1.tile([100, 1024], F32, tag="S1", name="S1a"),
                          ps_s1.tile([100, 1024], F32, tag="S1", name="S1b")]
                    for lo, hi in CH1:
                        nc.tensor.matmul(
                            Ss[0][:, lo:hi], ka3[:, :, q0, 0:N],
                            qb3[:, :, lo:hi], start=True, stop=True,
                            perf_mode=DR)
                        nc.tensor.matmul(
                            Ss[1][:, lo:hi], ka3[:, :, q1, 0:N],
                            qb3[:, :, lo:hi], start=True, stop=True,
                            perf_mode=DR)
                    Es = []
                    for S in Ss:
                        E = epool.tile([100, BN], F16, tag="E1")
                        nc.scalar.activation(E[:], S[:, 0:BN], AF.Exp,
                                             scale=0.125)
                        Es.append(E)
                    As = mpool.tile([128, BN], BF16, tag="As1")
                    for ci, (lo, hi) in enumerate(CH1):
                        Ac = (ps_a1.tile([128, 512], F32, tag="A1a", name="Aa")
                              if ci == 0 else
                              ps_a1.tile([128, 512], F32, tag="A1b", name="Ab"))
                        vaLs = (sb["vaL0"][:, 128 * j:128 * (j + 1)]
                                if j < 4 else
                                sb["vaLbig"][:, 128 * j - 512:
                                             128 * (j + 1) - 512])
                        vaRs = (sb["vaR0"][:, 128 * j:128 * (j + 1)]
                                if j < 4 else
                                sb["vaRbig"][:, 128 * j - 512:
                                             128 * (j + 1) - 512])
                        nc.tensor.matmul(Ac[:, 0:hi - lo], vaLs,
                                         Es[0][:, lo:hi],
                                         start=True, stop=False)
                        nc.tensor.matmul(Ac[:, 0:hi - lo], vaRs,
                                         Es[1][:, lo:hi],
                                         start=False, stop=True)
                        nc.vector.tensor_copy(As[:, lo:hi], Ac[:, 0:hi - lo])
                    nc.sync.dma_start(as1o[:, BN * j:BN * (j + 1)], As[:])



            # ---------------- path 2: per p over all (q n) -----------------
            with (
                tc.tile_pool(name="ps_s2", bufs=2, space="PSUM") as ps_s2,
                tc.tile_pool(name="ps_a2", bufs=int(os.environ.get("K_A2_BUFS", "2")), space="PSUM") as ps_a2,
            ):
                kb3 = sb["kbTdr"][:].rearrange("p (two b m) -> p two b m",
                                               two=2, b=PB)
                qa3 = sb["qaTdr"][:].rearrange("p (two n) -> p two n", two=2)
                for p in range(PB):
                    E2 = epool.tile([100, B * N], F16, tag="E2", bufs=E2_BUFS)
                    for lo, hi in E2CH:
                        S2 = ps_s2.tile([100, E2W], F32, tag="S2")
                        for l2 in range(lo, hi, 512):
                            h2 = min(l2 + 512, hi)
                            nc.tensor.matmul(
                                S2[:, l2 - lo:h2 - lo],
                                kb3[:, :, p, 0:N],
                                qa3[:, :, l2:h2],
                                start=True, stop=True, perf_mode=DR)
                        nc.scalar.activation(E2[:, lo:hi], S2[:, 0:hi - lo],
                                             AF.Exp, scale=0.125)
                    for g in range(4):  # groups of two chunk-pairs (1600 cols)
                        As2 = mpool.tile([128, 800], BF16, tag="As2", bufs=M2_BUFS)
                        for h in range(2):
                            j2 = 2 * g + h
                            c0 = 800 * j2
                            A2 = ps_a2.tile([128, 400], F32, tag="A2",
                                            name="A2")
                            nc.tensor.matmul(
                                A2[:], sb["vbL"][:, 128 * p:128 * (p + 1)],
                                E2[:, c0:c0 + 400], start=True, stop=False)
                            nc.tensor.matmul(
                                A2[:], sb["vbR"][:, 128 * p:128 * (p + 1)],
                                E2[:, c0 + 400:c0 + 800],
                                start=False, stop=True)
                            nc.vector.tensor_copy(As2[:, 400 * h:400 * (h + 1)],
                                                  A2[:])
                        nc.sync.dma_start(
                            as2o[:, 3200 * p + 800 * g:
                                 3200 * p + 800 * (g + 1)], As2[:])

    _split_multi_waits(nc)
    return nc


# ---------------------------------------------------------------- host

_progs = {}


def _install_compile_cache():
    """Persist compiled NEFF-wrapped custom calls across processes: walrus
    compilation takes tens of seconds per program and bass2jax recompiles
    in every fresh process otherwise."""
    import hashlib
    import pathlib
    from concourse import bass2jax
    if getattr(bass2jax, "_ant_disk_cache", False):
        return
    bass2jax._ant_disk_cache = True
    orig = bass2jax.neuronx_cc_hook
    cdir = pathlib.Path(os.environ.get("BASS_NEFF_CACHE",
                                       "/tmp/bass_neff_cache"))
    try:
        cdir.mkdir(parents=True, exist_ok=True)
    except OSError:
        return

    def cached_hook(code, code_format, platform_version, file_prefix):
        try:
            key = hashlib.sha256(
                bytes(code) + b"|" + bytes(code_format)).hexdigest()
            path = cdir / f"{key}.neffcall"
            if path.exists():
                return 0, path.read_bytes()
        except Exception:
            return orig(code, code_format, platform_version, file_prefix)
        rc, blob = orig(code, code_format, platform_version, file_prefix)
        if rc == 0:
            try:
                tmp = path.with_suffix(f".tmp{os.getpid()}")
                tmp.write_bytes(blob)
                tmp.rename(path)
            except OSError:
                pass
        return rc, blob

    bass2jax.neuronx_cc_hook = cached_hook
    try:
        import libneuronxla
        if libneuronxla.neuronx_cc is orig:
            libneuronxla.neuronx_cc = cached_hook
    except ImportError:
        pass


def _get_progs():
    if "p1" not in _progs:
        _install_compile_cache()
        _progs["p1"] = build_prog1()
        _progs["p2"] = build_prog2()
    return _progs["p1"], _progs["p2"]


def _masters():
    import ml_dtypes
    m1 = np.zeros((128, 320), ml_dtypes.bfloat16)
    m1[0:64, 128] = 1.0   # up-plane (rows 0:64 of rhs) -> out row q
    m1[64:128, 129] = 1.0  # down-plane -> out row q+1
    m8 = np.zeros((128, 320), ml_dtypes.bfloat16)
    m8[0:64, 128] = 1.0
    m8[64:128, 136] = 1.0  # down-plane -> out row r0+8
    return m1, m8


def _dr_pack_k(x, pad_to=None):
    """Pack [K, M] (K contraction, even) into DoubleRow layout
    [K//2, 2*M] fp8e4 with k = (K//2)*s + p."""
    import ml_dtypes
    K = x.shape[0]
    h = K // 2
    arr = x.reshape(2, h, *x.shape[1:]).transpose(1, 0, *range(2, x.ndim + 1))
    return np.ascontiguousarray(arr.reshape(h, -1).astype(
        ml_dtypes.float8_e4m3fn))


def _dr_pack_k_padded(x, nblk, blk, pad):
    """[K, nblk*blk] -> DR fp8 [K//2, 2*nblk*pad] with each blk padded."""
    import ml_dtypes
    K = x.shape[0]
    h = K // 2
    a = x.reshape(2, h, nblk, blk).transpose(1, 0, 2, 3)
    z = np.zeros((h, 2, nblk, pad), np.float32)
    z[:, :, :, 0:blk] = a
    return np.ascontiguousarray(z.reshape(h, -1).astype(
        ml_dtypes.float8_e4m3fn))


def kernel(features_a, features_b, Wq1, Wq2, Wk1, Wk2, Wv1, Wv2):
    import ml_dtypes
    nc1, nc2 = _get_progs()
    cc = np.ascontiguousarray

    fa = np.asarray(features_a, np.float32).reshape(B, C, N)
    fb = np.asarray(features_b, np.float32).reshape(B, C, N)

    def feat_dr(f_core):  # [PB, C, N] -> [KT1*64, 2*BN] fp8 DR
        fT = f_core.transpose(1, 0, 2).reshape(C, BN)
        a = fT.reshape(KT1, 2, 64, BN).transpose(0, 2, 1, 3)
        return cc(a.reshape(KT1 * 64, 2 * BN).astype(ml_dtypes.float8_e4m3fn))

    def w1_dr(W):  # [C, C] -> [KT1*64, 2*C] fp8 DR
        a = np.asarray(W, np.float32).reshape(KT1, 2, 64, C).transpose(
            0, 2, 1, 3)
        return cc(a.reshape(KT1 * 64, 2 * C).astype(ml_dtypes.float8_e4m3fn))

    ws = {"wq1dr": w1_dr(Wq1), "wk1dr": w1_dr(Wk1), "wv1dr": w1_dr(Wv1)}
    ws.update({k: cc(np.asarray(v, np.float32).astype(ml_dtypes.bfloat16))
               for k, v in (("wq2", Wq2), ("wk2", Wk2), ("wv2", Wv2))})

    in1 = [dict(fa8dr=feat_dr(fa[PB * i:PB * (i + 1)]),
                fb8dr=feat_dr(fb[PB * i:PB * (i + 1)]), **ws)
           for i in range(CORES)]
    res1 = run_bass_kernel_spmd(nc1, in1, core_ids=list(range(CORES)))

    def gather(name):
        return np.concatenate([res1.results[i][name] for i in range(CORES)],
                              axis=1)

    qaT, kaT, vaT = gather("qaT8"), gather("kaT8"), gather("vaT8")
    qbT = [res1.results[i]["qbT8"] for i in range(CORES)]
    kbT = [res1.results[i]["kbT8"] for i in range(CORES)]
    vbT = [res1.results[i]["vbT8"] for i in range(CORES)]

    # a-side derived tensors (shared by all cores)
    vaT32 = vaT.astype(np.float32)
    va_nm = cc(vaT.T)                       # [B*N, INNER] fp16
    na = np.maximum(np.sqrt((vaT32 * vaT32).sum(0)), EPS)
    vhat_aT = vaT32 / na[None, :]
    vaL = np.zeros((N, (B // 2) * 128), np.float16)
    vaR = np.zeros((N, (B // 2) * 128), np.float16)
    for j in range(B // 2):
        vaL[:, 128 * j:128 * j + 64] = va_nm[N * 2 * j:N * (2 * j + 1)]
        vaR[:, 128 * j + 64:128 * (j + 1)] = va_nm[N * (2 * j + 1):
                                                   N * (2 * j + 2)]
    vhat_aT2 = np.zeros((128, B * N // 2), np.float32)
    for j2 in range(8):
        vhat_aT2[0:64, 400 * j2:400 * (j2 + 1)] = \
            vhat_aT[:, 800 * j2:800 * j2 + 400]
        vhat_aT2[64:128, 400 * j2:400 * (j2 + 1)] = \
            vhat_aT[:, 800 * j2 + 400:800 * (j2 + 1)]
    m1, m8 = _masters()

    kaTdr = _dr_pack_k_padded(kaT.astype(np.float32), B, N, MP)
    qaTdr = _dr_pack_k(qaT.astype(np.float32))
    in2 = []
    vhat_bTs = []
    for i in range(CORES):
        vbT32 = vbT[i].astype(np.float32)
        vb_nm = cc(vbT[i].T)                # [BN, INNER] fp16
        nb = np.maximum(np.sqrt((vbT32 * vbT32).sum(0)), EPS)
        vhat_bT = vbT32 / nb[None, :]
        vbL = np.zeros((N, PB * 128), np.float16)
        vbR = np.zeros((N, PB * 128), np.float16)
        for p in range(PB):
            vbL[:, 128 * p:128 * p + 64] = vb_nm[N * p:N * (p + 1)]
            vbR[:, 128 * p + 64:128 * (p + 1)] = vb_nm[N * p:N * (p + 1)]
        vhat_bTs.append(vhat_bT)
        in2.append(dict(
            kaTdr=kaTdr, qaTdr=qaTdr,
            qbTdr=_dr_pack_k(qbT[i].astype(np.float32)),
            kbTdr=_dr_pack_k_padded(kbT[i].astype(np.float32), PB, N, MP),
            vaL=vaL, vaR=vaR, vbL=vbL, vbR=vbR))
    res2 = run_bass_kernel_spmd(nc2, in2, core_ids=list(range(CORES)))

    sim = np.zeros((B, B), np.float32)
    for i in range(CORES):
        r = res2.results[i]
        # path1: As1 col-block 800j = pair j (rows 0:64 -> q=2j,
        # rows 64:128 -> q=2j+1, cols (p, n)); dot/ny2 on host
        as1 = np.asarray(r["as1o"], np.float32).reshape(128, 32, 800)
        vb_h = vhat_bTs[i]                              # [64 i, 800 (p n)]
        ny2_1 = np.empty((64, 800), np.float32)
        dot1 = np.empty((64, 800), np.float32)
        ny2_1[0::2] = (as1[0:64] ** 2).sum(0)
        ny2_1[1::2] = (as1[64:128] ** 2).sum(0)
        dot1[0::2] = np.einsum('ijc,ic->jc', as1[0:64], vb_h)
        dot1[1::2] = np.einsum('ijc,ic->jc', as1[64:128], vb_h)
        cos1 = dot1 / np.maximum(np.sqrt(ny2_1), EPS)
        sim1 = cos1.reshape(64, PB, N).sum(-1)          # [q, p]

        # path2: As2 cols 3200p + 800g + 400h + c; rows 0:64 ->
        # qn = 800*(2g+h)+c, rows 64:128 -> +400; vhat_a [64, (g,h,half,c)]
        as2 = np.asarray(r["as2o"], np.float32).reshape(128, PB, 4, 2, 400)
        va4 = vhat_aT.reshape(64, 4, 2, 2, 400)         # [i, g, h, half, c]
        ny_lo = (as2[0:64] ** 2).sum(0).reshape(PB, 8, 400)
        ny_hi = (as2[64:128] ** 2).sum(0).reshape(PB, 8, 400)
        ny2_2 = np.concatenate([ny_lo, ny_hi], axis=2).reshape(PB, B * N)
        d_lo = np.einsum('ipghc,ighc->pghc', as2[0:64], va4[:, :, :, 0])
        d_hi = np.einsum('ipghc,ighc->pghc', as2[64:128], va4[:, :, :, 1])
        dot2 = np.concatenate([d_lo.reshape(PB, 8, 400),
                               d_hi.reshape(PB, 8, 400)],
                              axis=2).reshape(PB, B * N)
        cos2 = dot2 / np.maximum(np.sqrt(ny2_2), EPS)
        sim2 = cos2.reshape(PB, B, N).sum(-1)           # [p, q]

        sim[PB * i:PB * (i + 1)] = (sim1.T + sim2) / N
    return sim



# revision 76
# speedup vs baseline: 1.0008x; 1.0008x over previous
"""Trainium2 Bass kernel for nn_AttentionSimilarity.

Contract: kernel(**inputs) takes the FULL unsharded inputs (numpy) and
returns the FULL [64, 64] similarity matrix, distributing work across 8
NeuronCores internally.

Structure:
  prog1 (projections, sharded by batch): each core projects its 8
    a-batches and 8 b-batches through the three two-layer MLPs,
    emitting qaT/kaT/vaT/qbT/kbT/vbT chunks in [inner, (batch, n)]
    layout. Host gathers the a-side to full tensors.
  prog2 (attention, sharded by p = b-side batch): each core computes
    both attention paths for its 8 p's against all 64 q's, the cosine
    numerators/denominators via selector matmuls on the PE, and the
    per-(p,q) sums over n. Host assembles the [64, 64] output.

Math notes:
  - softmax feeds only cosine similarity, which is scale-invariant in
    the aligned vector, so the softmax max-shift and denominator cancel:
    softmax reduces to exp(scores/8).
  - the x-side cosine norm is folded on the host (vhat = v / max(|v|, eps)).
  - 1/max(|y|, eps) and the dot with vhat are applied on the host from
    the streamed-out aligned values.

Performance notes (vs the first working version):
  - prog1 W1 layer and both programs' score matmuls run in fp8e4 with
    MatmulPerfMode.DoubleRow (2 contraction rows per PE partition, 0.5
    cycles/output column): weights/features/q/k are DR-packed on the
    host ([K/2, 2, M] with k = (K/2)*s + p; lhsT m-blocks padded to
    MP=112 so the DR pair-stride stays 16-byte aligned).
  - the entire cosine stage (dot, squared-norm, rsqrt, mean over n)
    is computed on the HOST: the aligned values (As, bf16) stream out
    over the otherwise-idle DMA engines, deleting the M/SQ multiplies,
    all selector-reduce matmuls, the P1/P2 PSUM accumulators (freeing
    banks for aligned double-buffering), and the device epilogues.
    The device does projections, scores, softmax-exp and the aligned
    matmuls -- all of the O(B^2 N^2) compute.
  - warmup/tail: weight DMAs are split/consolidated so the first matmul
    starts as early as possible; prog1's W2 PSUM/copy/DMA pipeline is
    chunked per bank so stores drain during compute; path2 score tiles
    are 1536 columns (3 PSUM banks) to amortize the fixed per-
    instruction ACT access latency on the softmax exp, which is the
    saturated engine (~98% busy) in the final balance.
  - measured rel err vs fp32 reference: ~1.7e-3.

Dead end (measured): packing score tiles to 128 partitions by mixing
(q, m) across rows would cut exp columns 100/128, but the follow-up
aligned matmuls need operand slices at arbitrary partition offsets and
the PE requires base partition 0/32/64 (bass matmul assert); since
100 is not a multiple of 32, per-q slices of a packed layout are
unaddressable. The [m<=100, cols] score layout is forced.
"""

import os
import sys

sys.path.insert(0, "/opt/trn_rl_repo")
os.environ.setdefault("NEURON_RT_RESET_CORES", "1")

import numpy as np
import ml_dtypes  # noqa: F401  (bf16 host arrays)

import bass_rust
import concourse.bass as bass
import concourse.mybir as mybir
import concourse.tile as tile
from concourse.bass_utils import run_bass_kernel_spmd

F32 = mybir.dt.float32
F32R = mybir.dt.float32r
BF16 = mybir.dt.bfloat16
F16 = mybir.dt.float16
F8E4 = mybir.dt.float8e4
AF = mybir.ActivationFunctionType
DR = mybir.MatmulPerfMode.DoubleRow

B = 64          # batches per side
C = 512         # channels
N = 100         # H*W tokens per batch
INNER = 64      # projected dim
CORES = 8
PB = B // CORES  # batches per core (8)
BN = PB * N      # 800: (batch, n) columns per core chunk
EPS = 1e-8
KT1 = C // 128   # prog1 contraction tiles (4)
MP = 112         # fp8-DR padded m stride (112 % 16 == 0, >= N)

E1_BUFS = int(os.environ.get("K_E1_BUFS", "5"))
SEL_LAG = int(os.environ.get("K_SEL_LAG", "4"))
POOL_MOD1 = int(os.environ.get("K_POOL_MOD1", os.environ.get("K_POOL_MOD", "3")))
POOL_MOD2 = int(os.environ.get("K_POOL_MOD2", os.environ.get("K_POOL_MOD", "2")))
SEL_LAG2 = int(os.environ.get("K_SEL_LAG2", "4"))
M2_BUFS = int(os.environ.get("K_M2_BUFS", "8"))
MPOOL_MOD = int(os.environ.get("K_MPOOL_MOD", "0"))  # 0=never, k=every kth M on pool
M_BUFS = int(os.environ.get("K_M_BUFS", "8"))
E2_BUFS = int(os.environ.get("K_E2_BUFS", "3"))
S1_BUFS = int(os.environ.get("K_S1_BUFS", "2"))
A1_BUFS = int(os.environ.get("K_A1_BUFS", "1"))

_waitsplit_ctr = [0]


def _split_multi_waits(nc, max_waits=1):
    """This container's walrus build accepts at most ONE sync wait per
    instruction; Tile attaches several. Move extras onto preceding
    same-engine NoOps (engines are in-order, so semantics hold)."""
    n_split = 0
    for f in nc.m.functions:
        for blk in f.blocks:
            insts = list(blk.instructions)
            new_list = []
            changed = False
            for inst in insts:
                si = inst.sync_info
                waits = list(si.on_wait) if (si is not None and si.on_wait) else []
                if len(waits) > max_waits:
                    for w in waits[:-max_waits]:
                        _waitsplit_ctr[0] += 1
                        nop = mybir.InstNoOp(
                            name=f"I-waitsplit-{_waitsplit_ctr[0]}",
                            engine=inst.engine,
                            ins=[],
                            outs=[],
                            sync_info=bass_rust.SyncInfo(on_wait=[w], on_update=[]),
                        )
                        nc.register_instruction(nop, overwrite=True)
                        new_list.append(nop)
                        n_split += 1
                    si.on_wait = waits[-max_waits:]
                    inst.sync_info = si
                    changed = True
                new_list.append(inst)
            if changed:
                blk.instructions = new_list
    return n_split


# ---------------------------------------------------------------- prog1

def build_prog1():
    """Projection program. Per-core inputs:
      fa8dr, fb8dr: [KT*64, 2*BN] f8e4 DoubleRow-packed features
        (row kt*64+p, col (s, (b n)) holds feat[c = 128*kt + 64*s + p])
      wq1dr/...: [KT*64, 2*C] f8e4 DR weights (col (s, c_out))
      wq2/...: [C, INNER] bf16
    Outputs: qaT8/kaT8/vaT8/qbT8/kbT8/vbT8: [INNER, BN]  ([i, (b n)])
    """
    nc = bass.Bass("TRN2", target_bir_lowering=False, debug=False,
                   num_devices=CORES)
    fa8 = nc.dram_tensor("fa8dr", [KT1 * 64, 2 * BN], F8E4,
                         kind="ExternalInput").ap()
    fb8 = nc.dram_tensor("fb8dr", [KT1 * 64, 2 * BN], F8E4,
                         kind="ExternalInput").ap()
    w1 = {p: nc.dram_tensor(f"w{p}1dr", [KT1 * 64, 2 * C], F8E4,
                            kind="ExternalInput").ap()
          for p in "qkv"}
    w2 = {p: nc.dram_tensor(f"w{p}2", [C, INNER], BF16, kind="ExternalInput").ap()
          for p in "qkv"}
    outs = {(s, p): nc.dram_tensor(f"{p}{s}T8", [INNER, BN], F16,
                                   kind="ExternalOutput").ap()
            for s in "ab" for p in "qkv"}

    KT = KT1  # 4 contraction tiles of 128 (64 partitions x 2 DR)
    CT = C // 128  # 4 c_out tiles
    CH = [(0, 512), (512, BN)]  # psum-bank-aligned column chunks of BN

    with tile.TileContext(nc) as tc:
        with (
            tc.tile_pool(name="wpool", bufs=1) as wpool,
            tc.tile_pool(name="fpool", bufs=int(os.environ.get("K_F_BUFS", "2"))) as fpool,
            tc.tile_pool(name="hpool", bufs=int(os.environ.get("K_H_BUFS", "4"))) as hpool,
            tc.tile_pool(name="opool", bufs=int(os.environ.get("K_O_BUFS", "3"))) as opool,
            tc.tile_pool(name="psH", bufs=int(os.environ.get("K_PSH_BUFS", "3")), space="PSUM") as psHp,
            tc.tile_pool(name="psO", bufs=int(os.environ.get("K_PSO_BUFS", "1")), space="PSUM") as psOp,
        ):
            w1sb, w2sb = {}, {}

            def load_w1(p):
                wt = wpool.tile([64, KT * 2 * C], F8E4, tag=f"w1{p}",
                                name=f"w1{p}sb")
                wv = wt[:].rearrange("p (kt x) -> p kt x", kt=KT)
                dv = w1[p].rearrange("(kt p) x -> p kt x", p=64)
                nc.sync.dma_start(wv[:, 0:2], dv[:, 0:2])
                nc.sync.dma_start(wv[:, 2:KT], dv[:, 2:KT])
                w1sb[p] = wt

            def load_w(p):
                load_w1(p)
                w2sb[p] = wpool.tile([128, KT * INNER], BF16, tag=f"w2{p}",
                                     name=f"w2{p}sb")
                nc.sync.dma_start(
                    w2sb[p][:].rearrange("p (kt i) -> p kt i", kt=KT),
                    w2[p].rearrange("(kt p) i -> p kt i", p=128))

            for s, feat in (("a", fa8), ("b", fb8)):
                fts = []
                for kt in range(KT):
                    if s == "a" and kt == 0:
                        load_w1("q")
                    ft = fpool.tile([64, 2 * BN], F8E4, tag=f"f{kt}")
                    nc.sync.dma_start(ft[:], feat[64 * kt:64 * (kt + 1), :])
                    fts.append(ft)
                if s == "a":
                    w2sb["q"] = wpool.tile([128, KT * INNER], BF16, tag="w2q",
                                           name="w2qsb")
                    nc.sync.dma_start(
                        w2sb["q"][:].rearrange("p (kt i) -> p kt i", kt=KT),
                        w2["q"].rearrange("(kt p) i -> p kt i", p=128))
                    load_w("k")
                    load_w("v")
                for p in "qkv":
                    hts = []
                    for t in range(CT):
                        psH = psHp.tile([128, 1024], F32, tag="psH")
                        for lo, hi in CH:
                            for kt in range(KT):
                                nc.tensor.matmul(
                                    psH[:, lo:hi],
                                    w1sb[p][:].rearrange(
                                        "p (kt two c) -> p kt two c",
                                        kt=KT, two=2)[
                                        :, kt, :, 128 * t:128 * t + 128],
                                    fts[kt][:].rearrange(
                                        "p (two n) -> p two n", two=2)[
                                        :, :, lo:hi],
                                    start=(kt == 0), stop=(kt == KT - 1),
                                    perf_mode=mybir.MatmulPerfMode.DoubleRow)
                        ht = hpool.tile([128, BN], BF16, tag=f"h{t}")
                        if t % 2 == 0:
                            nc.scalar.activation(ht[:], psH[:, 0:BN], AF.Relu)
                        else:
                            nc.vector.tensor_scalar_max(ht[:], psH[:, 0:BN],
                                                        0.0)
                        hts.append(ht)
                    psOs = [psOp.tile([INNER, 512], F32, tag="psOa",
                                      name="psOa"),
                            psOp.tile([INNER, 512], F32, tag="psOb",
                                      name="psOb")]
                    ot = opool.tile([INNER, BN], F16, tag="out")
                    for ci, (lo, hi) in enumerate(CH):
                        for kt in range(KT):
                            nc.tensor.matmul(
                                psOs[ci][:, 0:hi - lo],
                                w2sb[p][:, INNER * kt:INNER * (kt + 1)],
                                hts[kt][:, lo:hi],
                                start=(kt == 0), stop=(kt == KT - 1))
                        nc.scalar.copy(ot[:, lo:hi], psOs[ci][:, 0:hi - lo])
                        nc.sync.dma_start(outs[(s, p)][:, lo:hi],
                                          ot[:, lo:hi])

    _split_multi_waits(nc)
    return nc


# ---------------------------------------------------------------- prog2

def build_prog2():
    """Attention program, sharded over p (this core's 8 b-batches).

    Inputs (f32r unless noted):
      kaT, qaT      [INNER, B*N]   a-side K^T / Q^T, i on partitions
      qbT, kbT      [INNER, BN]    this core's b-side chunks
      vaL, vaR      [N, 32*128]    per q-pair j: [va[2j] | 0], [0 | va[2j+1]]
      vbL, vbR      [N, 8*128]     per p: [vb[p] | 0], [0 | vb[p]]
      vhat_bT2 f32  [128, BN]      v̂b^T twice (rows 0:64 and 64:128)
      vhat_aT2 f32  [128, B*N//2]  v̂a^T in chunk-pair layout
      master1, master8 [128, 320]  reduce-selector constants
    Outputs (f32):
      out1 [64, PB]   path1 per-(q, p) sums over n of cos1
      out2 [128, 4]   path2 sums; row r: chunk=r//8, p=r%8; q=4*(r//8)+col
    """
    nc = bass.Bass("TRN2", target_bir_lowering=False, debug=False,
                   num_devices=CORES)
    din = {}
    for name, shape, dt in [
        ("kaTdr", [32, 2 * B * MP], F8E4), ("qaTdr", [32, 2 * B * N], F8E4),
        ("qbTdr", [32, 2 * BN], F8E4), ("kbTdr", [32, 2 * PB * MP], F8E4),
        ("vaL", [N, (B // 2) * 128], F16), ("vaR", [N, (B // 2) * 128], F16),
        ("vbL", [N, PB * 128], F16), ("vbR", [N, PB * 128], F16),
    ]:
        din[name] = nc.dram_tensor(name, shape, dt, kind="ExternalInput").ap()
    as1o = nc.dram_tensor("as1o", [128, 32 * BN], BF16,
                          kind="ExternalOutput").ap()
    as2o = nc.dram_tensor("as2o", [128, 32 * 800], BF16,
                          kind="ExternalOutput").ap()

    CH1 = [(0, 512), (512, BN)]          # path1 column chunks of (p n)
    E2W = 1536                           # path2 score-chunk width (3 banks)
    NQ2 = (B * N) // E2W                 # 4 full chunks + 256 remainder
    E2CH = [(E2W * j, min(E2W * (j + 1), B * N)) for j in range(NQ2 + 1)]

    with tile.TileContext(nc) as tc:
        from contextlib import ExitStack
        with ExitStack() as ctx:
            inp = ctx.enter_context(tc.tile_pool(name="inp", bufs=1))
            sb = {}

            def load(name, ap=None, cols=None, cname=None):
                ap = din[name] if ap is None else ap
                if cols is not None:
                    ap = ap[:, cols[0]:cols[1]]
                cname = cname or name
                t = inp.tile(list(ap.shape), ap.dtype, tag=cname,
                             name=f"sb_{cname}")
                nc.sync.dma_start(t[:], ap[:])
                sb[cname] = t

            # path1-critical tensors first (chunked so compute starts early)
            # kaTdr in 4 q-chunks of 16 (both DR halves per chunk)
            ka_t = inp.tile([32, 2 * B * MP], F8E4, tag="kaTdr",
                            name="sb_kaTdr")
            sb["kaTdr"] = ka_t
            ka3d = din["kaTdr"].rearrange("p (two q m) -> p two q m",
                                          two=2, q=B)
            ka3s = ka_t[:].rearrange("p (two q m) -> p two q m", two=2, q=B)
            nc.sync.dma_start(ka3s[:, :, 0:16, :], ka3d[:, :, 0:16, :])
            load("qbTdr")
            load("vaL", cols=(0, 512), cname="vaL0")
            load("vaR", cols=(0, 512), cname="vaR0")
            for c in range(1, 4):
                nc.sync.dma_start(ka3s[:, :, 16 * c:16 * (c + 1), :],
                                  ka3d[:, :, 16 * c:16 * (c + 1), :])
            # vaL/vaR tail in 2 big chunks each, sliced into per-512 views
            for nm in ("vaL", "vaR"):
                big = inp.tile([N, 3584], F16, tag=f"{nm}big",
                               name=f"sb_{nm}big")
                nc.sync.dma_start(big[:, 0:1536], din[nm][:, 512:2048])
                nc.sync.dma_start(big[:, 1536:3584], din[nm][:, 2048:4096])
                for c in range(1, 8):
                    sb[f"{nm}{c}"] = None
                sb[f"{nm}big"] = big
            # qaTdr in 2 n-chunks (both DR halves per chunk)
            qa_t = inp.tile([32, 2 * B * N], F8E4, tag="qaTdr",
                            name="sb_qaTdr")
            sb["qaTdr"] = qa_t
            qa3d = din["qaTdr"].rearrange("p (two n) -> p two n", two=2)
            qa3s = qa_t[:].rearrange("p (two n) -> p two n", two=2)
            nc.sync.dma_start(qa3s[:, :, 0:3200], qa3d[:, :, 0:3200])
            nc.sync.dma_start(qa3s[:, :, 3200:6400], qa3d[:, :, 3200:6400])
            for name in ("kbTdr", "vbL", "vbR"):
                load(name)

            epool = ctx.enter_context(tc.tile_pool(name="epool", bufs=E1_BUFS))
            mpool = ctx.enter_context(tc.tile_pool(name="mpool", bufs=M_BUFS))
            fin = ctx.enter_context(tc.tile_pool(name="fin", bufs=1))

            # ---------------- path 1: per q-pair over this core's (p n) ----
            with (
                tc.tile_pool(name="ps_s1", bufs=S1_BUFS, space="PSUM") as ps_s1,
                tc.tile_pool(name="ps_a1", bufs=int(os.environ.get("K_A1_BUFS","2")), space="PSUM") as ps_a1,
            ):
                ka3 = sb["kaTdr"][:].rearrange("p (two q m) -> p two q m",
                                               two=2, q=B)
                qb3 = sb["qbTdr"][:].rearrange("p (two n) -> p two n", two=2)
                for j in range(B // 2):
                    q0, q1 = 2 * j, 2 * j + 1
                    Ss = [ps_s1.tile([100, 1024], F32, tag="S1", name="S1a"),
                          ps_s1.tile([100, 1024], F32, tag="S1", name="S1b")]
                    for lo, hi in CH1:
                        nc.tensor.matmul(
                            Ss[0][:, lo:hi], ka3[:, :, q0, 0:N],
                            qb3[:, :, lo:hi], start=True, stop=True,
                            perf_mode=DR)
                        nc.tensor.matmul(
                            Ss[1][:, lo:hi], ka3[:, :, q1, 0:N],
                            qb3[:, :, lo:hi], start=True, stop=True,
                            perf_mode=DR)
                    Es = []
                    for S in Ss:
                        E = epool.tile([100, BN], F16, tag="E1")
                        nc.scalar.activation(E[:], S[:, 0:BN], AF.Exp,
                                             scale=0.125)
                        Es.append(E)
                    As = mpool.tile([128, BN], BF16, tag="As1")
                    for ci, (lo, hi) in enumerate(CH1):
                        Ac = (ps_a1.tile([128, 512], F32, tag="A1a", name="Aa")
                              if ci == 0 else
                              ps_a1.tile([128, 512], F32, tag="A1b", name="Ab"))
                        vaLs = (sb["vaL0"][:, 128 * j:128 * (j + 1)]
                                if j < 4 else
                                sb["vaLbig"][:, 128 * j - 512:
                                             128 * (j + 1) - 512])
                        vaRs = (sb["vaR0"][:, 128 * j:128 * (j + 1)]
                                if j < 4 else
                                sb["vaRbig"][:, 128 * j - 512:
                                             128 * (j + 1) - 512])
                        nc.tensor.matmul(Ac[:, 0:hi - lo], vaLs,
                                         Es[0][:, lo:hi],
                                         start=True, stop=False)
                        nc.tensor.matmul(Ac[:, 0:hi - lo], vaRs,
                                         Es[1][:, lo:hi],
                                         start=False, stop=True)
                        nc.vector.tensor_copy(As[:, lo:hi], Ac[:, 0:hi - lo])
                    nc.sync.dma_start(as1o[:, BN * j:BN * (j + 1)], As[:])



            # ---------------- path 2: per p over all (q n) -----------------
            with (
                tc.tile_pool(name="ps_s2", bufs=2, space="PSUM") as ps_s2,
                tc.tile_pool(name="ps_a2", bufs=int(os.environ.get("K_A2_BUFS", "2")), space="PSUM") as ps_a2,
            ):
                kb3 = sb["kbTdr"][:].rearrange("p (two b m) -> p two b m",
                                               two=2, b=PB)
                qa3 = sb["qaTdr"][:].rearrange("p (two n) -> p two n", two=2)
                for p in range(PB):
                    E2 = epool.tile([100, B * N], F16, tag="E2", bufs=E2_BUFS)
                    for lo, hi in E2CH:
                        S2 = ps_s2.tile([100, E2W], F32, tag="S2")
                        for l2 in range(lo, hi, 512):
                            h2 = min(l2 + 512, hi)
                            nc.tensor.matmul(
                                S2[:, l2 - lo:h2 - lo],
                                kb3[:, :, p, 0:N],
                                qa3[:, :, l2:h2],
                                start=True, stop=True, perf_mode=DR)
                        nc.scalar.activation(E2[:, lo:hi], S2[:, 0:hi - lo],
                                             AF.Exp, scale=0.125)
                    for g in range(4):  # groups of two chunk-pairs (1600 cols)
                        As2 = mpool.tile([128, 800], BF16, tag="As2", bufs=M2_BUFS)
                        for h in range(2):
                            j2 = 2 * g + h
                            c0 = 800 * j2
                            A2 = ps_a2.tile([128, 400], F32, tag="A2",
                                            name="A2")
                            nc.tensor.matmul(
                                A2[:], sb["vbL"][:, 128 * p:128 * (p + 1)],
                                E2[:, c0:c0 + 400], start=True, stop=False)
                            nc.tensor.matmul(
                                A2[:], sb["vbR"][:, 128 * p:128 * (p + 1)],
                                E2[:, c0 + 400:c0 + 800],
                                start=False, stop=True)
                            nc.vector.tensor_copy(As2[:, 400 * h:400 * (h + 1)],
                                                  A2[:])
                        nc.sync.dma_start(
                            as2o[:, 3200 * p + 800 * g:
                                 3200 * p + 800 * (g + 1)], As2[:])

    _split_multi_waits(nc)
    return nc


# ---------------------------------------------------------------- host

_progs = {}


def _install_compile_cache():
    """Persist compiled NEFF-wrapped custom calls across processes: walrus
    compilation takes tens of seconds per program and bass2jax recompiles
    in every fresh process otherwise."""
    import hashlib
    import pathlib
    from concourse import bass2jax
    if getattr(bass2jax, "_ant_disk_cache", False):
        return
    bass2jax._ant_disk_cache = True
    orig = bass2jax.neuronx_cc_hook
    cdir = pathlib.Path(os.environ.get("BASS_NEFF_CACHE",
                                       "/tmp/bass_neff_cache"))
    try:
        cdir.mkdir(parents=True, exist_ok=True)
    except OSError:
        return

    def cached_hook(code, code_format, platform_version, file_prefix):
        try:
            key = hashlib.sha256(
                bytes(code) + b"|" + bytes(code_format)).hexdigest()
            path = cdir / f"{key}.neffcall"
            if path.exists():
                return 0, path.read_bytes()
        except Exception:
            return orig(code, code_format, platform_version, file_prefix)
        rc, blob = orig(code, code_format, platform_version, file_prefix)
        if rc == 0:
            try:
                tmp = path.with_suffix(f".tmp{os.getpid()}")
                tmp.write_bytes(blob)
                tmp.rename(path)
            except OSError:
                pass
        return rc, blob

    bass2jax.neuronx_cc_hook = cached_hook
    try:
        import libneuronxla
        if libneuronxla.neuronx_cc is orig:
            libneuronxla.neuronx_cc = cached_hook
    except ImportError:
        pass


def _get_progs():
    if "p1" not in _progs:
        _install_compile_cache()
        _progs["p1"] = build_prog1()
        _progs["p2"] = build_prog2()
    return _progs["p1"], _progs["p2"]


def _masters():
    import ml_dtypes
    m1 = np.zeros((128, 320), ml_dtypes.bfloat16)
    m1[0:64, 128] = 1.0   # up-plane (rows 0:64 of rhs) -> out row q
    m1[64:128, 129] = 1.0  # down-plane -> out row q+1
    m8 = np.zeros((128, 320), ml_dtypes.bfloat16)
    m8[0:64, 128] = 1.0
    m8[64:128, 136] = 1.0  # down-plane -> out row r0+8
    return m1, m8


def _dr_pack_k(x, pad_to=None):
    """Pack [K, M] (K contraction, even) into DoubleRow layout
    [K//2, 2*M] fp8e4 with k = (K//2)*s + p."""
    import ml_dtypes
    K = x.shape[0]
    h = K // 2
    arr = x.reshape(2, h, *x.shape[1:]).transpose(1, 0, *range(2, x.ndim + 1))
    return np.ascontiguousarray(arr.reshape(h, -1).astype(
        ml_dtypes.float8_e4m3fn))


def _dr_pack_k_padded(x, nblk, blk, pad):
    """[K, nblk*blk] -> DR fp8 [K//2, 2*nblk*pad] with each blk padded."""
    import ml_dtypes
    K = x.shape[0]
    h = K // 2
    a = x.reshape(2, h, nblk, blk).transpose(1, 0, 2, 3)
    z = np.zeros((h, 2, nblk, pad), np.float32)
    z[:, :, :, 0:blk] = a
    return np.ascontiguousarray(z.reshape(h, -1).astype(
        ml_dtypes.float8_e4m3fn))


def kernel(features_a, features_b, Wq1, Wq2, Wk1, Wk2, Wv1, Wv2):
    import ml_dtypes
    nc1, nc2 = _get_progs()
    cc = np.ascontiguousarray

    fa = np.asarray(features_a, np.float32).reshape(B, C, N)
    fb = np.asarray(features_b, np.float32).reshape(B, C, N)

    def feat_dr(f_core):  # [PB, C, N] -> [KT1*64, 2*BN] fp8 DR
        fT = f_core.transpose(1, 0, 2).reshape(C, BN)
        a = fT.reshape(KT1, 2, 64, BN).transpose(0, 2, 1, 3)
        return cc(a.reshape(KT1 * 64, 2 * BN).astype(ml_dtypes.float8_e4m3fn))

    def w1_dr(W):  # [C, C] -> [KT1*64, 2*C] fp8 DR
        a = np.asarray(W, np.float32).reshape(KT1, 2, 64, C).transpose(
            0, 2, 1, 3)
        return cc(a.reshape(KT1 * 64, 2 * C).astype(ml_dtypes.float8_e4m3fn))

    ws = {"wq1dr": w1_dr(Wq1), "wk1dr": w1_dr(Wk1), "wv1dr": w1_dr(Wv1)}
    ws.update({k: cc(np.asarray(v, np.float32).astype(ml_dtypes.bfloat16))
               for k, v in (("wq2", Wq2), ("wk2", Wk2), ("wv2", Wv2))})

    in1 = [dict(fa8dr=feat_dr(fa[PB * i:PB * (i + 1)]),
                fb8dr=feat_dr(fb[PB * i:PB * (i + 1)]), **ws)
           for i in range(CORES)]
    res1 = run_bass_kernel_spmd(nc1, in1, core_ids=list(range(CORES)))

    def gather(name):
        return np.concatenate([res1.results[i][name] for i in range(CORES)],
                              axis=1)

    qaT, kaT, vaT = gather("qaT8"), gather("kaT8"), gather("vaT8")
    qbT = [res1.results[i]["qbT8"] for i in range(CORES)]
    kbT = [res1.results[i]["kbT8"] for i in range(CORES)]
    vbT = [res1.results[i]["vbT8"] for i in range(CORES)]

    # a-side derived tensors (shared by all cores)
    vaT32 = vaT.astype(np.float32)
    va_nm = cc(vaT.T)                       # [B*N, INNER] fp16
    na = np.maximum(np.sqrt((vaT32 * vaT32).sum(0)), EPS)
    vhat_aT = vaT32 / na[None, :]
    vaL = np.zeros((N, (B // 2) * 128), np.float16)
    vaR = np.zeros((N, (B // 2) * 128), np.float16)
    for j in range(B // 2):
        vaL[:, 128 * j:128 * j + 64] = va_nm[N * 2 * j:N * (2 * j + 1)]
        vaR[:, 128 * j + 64:128 * (j + 1)] = va_nm[N * (2 * j + 1):
                                                   N * (2 * j + 2)]
    vhat_aT2 = np.zeros((128, B * N // 2), np.float32)
    for j2 in range(8):
        vhat_aT2[0:64, 400 * j2:400 * (j2 + 1)] = \
            vhat_aT[:, 800 * j2:800 * j2 + 400]
        vhat_aT2[64:128, 400 * j2:400 * (j2 + 1)] = \
            vhat_aT[:, 800 * j2 + 400:800 * (j2 + 1)]
    m1, m8 = _masters()

    kaTdr = _dr_pack_k_padded(kaT.astype(np.float32), B, N, MP)
    qaTdr = _dr_pack_k(qaT.astype(np.float32))
    in2 = []
    vhat_bTs = []
    for i in range(CORES):
        vbT32 = vbT[i].astype(np.float32)
        vb_nm = cc(vbT[i].T)                # [BN, INNER] fp16
        nb = np.maximum(np.sqrt((vbT32 * vbT32).sum(0)), EPS)
        vhat_bT = vbT32 / nb[None, :]
        vbL = np.zeros((N, PB * 128), np.float16)
        vbR = np.zeros((N, PB * 128), np.float16)
        for p in range(PB):
            vbL[:, 128 * p:128 * p + 64] = vb_nm[N * p:N * (p + 1)]
            vbR[:, 128 * p + 64:128 * (p + 1)] = vb_nm[N * p:N * (p + 1)]
        vhat_bTs.append(vhat_bT)
        in2.append(dict(
            kaTdr=kaTdr, qaTdr=qaTdr,
            qbTdr=_dr_pack_k(qbT[i].astype(np.float32)),
            kbTdr=_dr_pack_k_padded(kbT[i].astype(np.float32), PB, N, MP),
            vaL=vaL, vaR=vaR, vbL=vbL, vbR=vbR))
    res2 = run_bass_kernel_spmd(nc2, in2, core_ids=list(range(CORES)))

    sim = np.zeros((B, B), np.float32)
    for i in range(CORES):
        r = res2.results[i]
        # path1: As1 col-block 800j = pair j (rows 0:64 -> q=2j,
        # rows 64:128 -> q=2j+1, cols (p, n)); dot/ny2 on host
        as1 = np.asarray(r["as1o"], np.float32).reshape(128, 32, 800)
        vb_h = vhat_bTs[i]                              # [64 i, 800 (p n)]
        ny2_1 = np.empty((64, 800), np.float32)
        dot1 = np.empty((64, 800), np.float32)
        ny2_1[0::2] = (as1[0:64] ** 2).sum(0)
        ny2_1[1::2] = (as1[64:128] ** 2).sum(0)
        dot1[0::2] = np.einsum('ijc,ic->jc', as1[0:64], vb_h)
        dot1[1::2] = np.einsum('ijc,ic->jc', as1[64:128], vb_h)
        cos1 = dot1 / np.maximum(np.sqrt(ny2_1), EPS)
        sim1 = cos1.reshape(64, PB, N).sum(-1)          # [q, p]

        # path2: As2 cols 3200p + 800g + 400h + c; rows 0:64 ->
        # qn = 800*(2g+h)+c, rows 64:128 -> +400; vhat_a [64, (g,h,half,c)]
        as2 = np.asarray(r["as2o"], np.float32).reshape(128, PB, 4, 2, 400)
        va4 = vhat_aT.reshape(64, 4, 2, 2, 400)         # [i, g, h, half, c]
        ny_lo = (as2[0:64] ** 2).sum(0).reshape(PB, 8, 400)
        ny_hi = (as2[64:128] ** 2).sum(0).reshape(PB, 8, 400)
        ny2_2 = np.concatenate([ny_lo, ny_hi], axis=2).reshape(PB, B * N)
        d_lo = np.einsum('ipghc,ighc->pghc', as2[0:64], va4[:, :, :, 0])
        d_hi = np.einsum('ipghc,ighc->pghc', as2[64:128], va4[:, :, :, 1])
        dot2 = np.concatenate([d_lo.reshape(PB, 8, 400),
                               d_hi.reshape(PB, 8, 400)],
                              axis=2).reshape(PB, B * N)
        cos2 = dot2 / np.maximum(np.sqrt(ny2_2), EPS)
        sim2 = cos2.reshape(PB, B, N).sum(-1)           # [p, q]

        sim[PB * i:PB * (i + 1)] = (sim1.T + sim2) / N
    return sim



# revision 77
# speedup vs baseline: 1.0023x; 1.0015x over previous
"""Trainium2 Bass kernel for nn_AttentionSimilarity.

Contract: kernel(**inputs) takes the FULL unsharded inputs (numpy) and
returns the FULL [64, 64] similarity matrix, distributing work across 8
NeuronCores internally.

Structure:
  prog1 (projections, sharded by batch): each core projects its 8
    a-batches and 8 b-batches through the three two-layer MLPs,
    emitting qaT/kaT/vaT/qbT/kbT/vbT chunks in [inner, (batch, n)]
    layout. Host gathers the a-side to full tensors.
  prog2 (attention, sharded by p = b-side batch): each core computes
    both attention paths for its 8 p's against all 64 q's, the cosine
    numerators/denominators via selector matmuls on the PE, and the
    per-(p,q) sums over n. Host assembles the [64, 64] output.

Math notes:
  - softmax feeds only cosine similarity, which is scale-invariant in
    the aligned vector, so the softmax max-shift and denominator cancel:
    softmax reduces to exp(scores/8).
  - the x-side cosine norm is folded on the host (vhat = v / max(|v|, eps)).
  - 1/max(|y|, eps) and the dot with vhat are applied on the host from
    the streamed-out aligned values.

Performance notes (vs the first working version):
  - prog1 W1 layer and both programs' score matmuls run in fp8e4 with
    MatmulPerfMode.DoubleRow (2 contraction rows per PE partition, 0.5
    cycles/output column): weights/features/q/k are DR-packed on the
    host ([K/2, 2, M] with k = (K/2)*s + p; lhsT m-blocks padded to
    MP=112 so the DR pair-stride stays 16-byte aligned).
  - the entire cosine stage (dot, squared-norm, rsqrt, mean over n)
    is computed on the HOST: the aligned values (As, bf16) stream out
    over the otherwise-idle DMA engines, deleting the M/SQ multiplies,
    all selector-reduce matmuls, the P1/P2 PSUM accumulators (freeing
    banks for aligned double-buffering), and the device epilogues.
    The device does projections, scores, softmax-exp and the aligned
    matmuls -- all of the O(B^2 N^2) compute.
  - warmup/tail: weight DMAs are split/consolidated so the first matmul
    starts as early as possible; prog1's W2 PSUM/copy/DMA pipeline is
    chunked per bank so stores drain during compute; path2 score tiles
    are 1536 columns (3 PSUM banks) to amortize the fixed per-
    instruction ACT access latency on the softmax exp, which is the
    saturated engine (~98% busy) in the final balance.
  - measured rel err vs fp32 reference: ~1.7e-3.

Dead end (measured): packing score tiles to 128 partitions by mixing
(q, m) across rows would cut exp columns 100/128, but the follow-up
aligned matmuls need operand slices at arbitrary partition offsets and
the PE requires base partition 0/32/64 (bass matmul assert); since
100 is not a multiple of 32, per-q slices of a packed layout are
unaddressable. The [m<=100, cols] score layout is forced.
"""

import os
import sys

sys.path.insert(0, "/opt/trn_rl_repo")
os.environ.setdefault("NEURON_RT_RESET_CORES", "1")

import numpy as np
import ml_dtypes  # noqa: F401  (bf16 host arrays)

import bass_rust
import concourse.bass as bass
import concourse.mybir as mybir
import concourse.tile as tile
from concourse.bass_utils import run_bass_kernel_spmd

F32 = mybir.dt.float32
F32R = mybir.dt.float32r
BF16 = mybir.dt.bfloat16
F16 = mybir.dt.float16
F8E4 = mybir.dt.float8e4
AF = mybir.ActivationFunctionType
DR = mybir.MatmulPerfMode.DoubleRow

B = 64          # batches per side
C = 512         # channels
N = 100         # H*W tokens per batch
INNER = 64      # projected dim
CORES = 8
PB = B // CORES  # batches per core (8)
BN = PB * N      # 800: (batch, n) columns per core chunk
EPS = 1e-8
KT1 = C // 128   # prog1 contraction tiles (4)
MP = 112         # fp8-DR padded m stride (112 % 16 == 0, >= N)

E1_BUFS = int(os.environ.get("K_E1_BUFS", "5"))
SEL_LAG = int(os.environ.get("K_SEL_LAG", "4"))
POOL_MOD1 = int(os.environ.get("K_POOL_MOD1", os.environ.get("K_POOL_MOD", "3")))
POOL_MOD2 = int(os.environ.get("K_POOL_MOD2", os.environ.get("K_POOL_MOD", "2")))
SEL_LAG2 = int(os.environ.get("K_SEL_LAG2", "4"))
M2_BUFS = int(os.environ.get("K_M2_BUFS", "8"))
MPOOL_MOD = int(os.environ.get("K_MPOOL_MOD", "0"))  # 0=never, k=every kth M on pool
M_BUFS = int(os.environ.get("K_M_BUFS", "8"))
E2_BUFS = int(os.environ.get("K_E2_BUFS", "3"))
S1_BUFS = int(os.environ.get("K_S1_BUFS", "2"))
A1_BUFS = int(os.environ.get("K_A1_BUFS", "1"))

_waitsplit_ctr = [0]


def _split_multi_waits(nc, max_waits=1):
    """This container's walrus build accepts at most ONE sync wait per
    instruction; Tile attaches several. Move extras onto preceding
    same-engine NoOps (engines are in-order, so semantics hold)."""
    n_split = 0
    for f in nc.m.functions:
        for blk in f.blocks:
            insts = list(blk.instructions)
            new_list = []
            changed = False
            for inst in insts:
                si = inst.sync_info
                waits = list(si.on_wait) if (si is not None and si.on_wait) else []
                if len(waits) > max_waits:
                    for w in waits[:-max_waits]:
                        _waitsplit_ctr[0] += 1
                        nop = mybir.InstNoOp(
                            name=f"I-waitsplit-{_waitsplit_ctr[0]}",
                            engine=inst.engine,
                            ins=[],
                            outs=[],
                            sync_info=bass_rust.SyncInfo(on_wait=[w], on_update=[]),
                        )
                        nc.register_instruction(nop, overwrite=True)
                        new_list.append(nop)
                        n_split += 1
                    si.on_wait = waits[-max_waits:]
                    inst.sync_info = si
                    changed = True
                new_list.append(inst)
            if changed:
                blk.instructions = new_list
    return n_split


# ---------------------------------------------------------------- prog1

def build_prog1():
    """Projection program. Per-core inputs:
      fa8dr, fb8dr: [KT*64, 2*BN] f8e4 DoubleRow-packed features
        (row kt*64+p, col (s, (b n)) holds feat[c = 128*kt + 64*s + p])
      wq1dr/...: [KT*64, 2*C] f8e4 DR weights (col (s, c_out))
      wq2/...: [C, INNER] bf16
    Outputs: qaT8/kaT8/vaT8/qbT8/kbT8/vbT8: [INNER, BN]  ([i, (b n)])
    """
    nc = bass.Bass("TRN2", target_bir_lowering=False, debug=False,
                   num_devices=CORES)
    fa8 = nc.dram_tensor("fa8dr", [KT1 * 64, 2 * BN], F8E4,
                         kind="ExternalInput").ap()
    fb8 = nc.dram_tensor("fb8dr", [KT1 * 64, 2 * BN], F8E4,
                         kind="ExternalInput").ap()
    w1 = {p: nc.dram_tensor(f"w{p}1dr", [KT1 * 64, 2 * C], F8E4,
                            kind="ExternalInput").ap()
          for p in "qkv"}
    w2 = {p: nc.dram_tensor(f"w{p}2", [C, INNER], BF16, kind="ExternalInput").ap()
          for p in "qkv"}
    outs = {(s, p): nc.dram_tensor(f"{p}{s}T8", [INNER, BN], F16,
                                   kind="ExternalOutput").ap()
            for s in "ab" for p in "qkv"}

    KT = KT1  # 4 contraction tiles of 128 (64 partitions x 2 DR)
    CT = C // 128  # 4 c_out tiles
    CH = [(0, 512), (512, BN)]  # psum-bank-aligned column chunks of BN

    with tile.TileContext(nc) as tc:
        with (
            tc.tile_pool(name="wpool", bufs=1) as wpool,
            tc.tile_pool(name="fpool", bufs=int(os.environ.get("K_F_BUFS", "3"))) as fpool,
            tc.tile_pool(name="hpool", bufs=int(os.environ.get("K_H_BUFS", "5"))) as hpool,
            tc.tile_pool(name="opool", bufs=int(os.environ.get("K_O_BUFS", "4"))) as opool,
            tc.tile_pool(name="psH", bufs=int(os.environ.get("K_PSH_BUFS", "3")), space="PSUM") as psHp,
            tc.tile_pool(name="psO", bufs=int(os.environ.get("K_PSO_BUFS", "1")), space="PSUM") as psOp,
        ):
            w1sb, w2sb = {}, {}

            def load_w1(p):
                wt = wpool.tile([64, KT * 2 * C], F8E4, tag=f"w1{p}",
                                name=f"w1{p}sb")
                wv = wt[:].rearrange("p (kt x) -> p kt x", kt=KT)
                dv = w1[p].rearrange("(kt p) x -> p kt x", p=64)
                nc.sync.dma_start(wv[:, 0:2], dv[:, 0:2])
                nc.sync.dma_start(wv[:, 2:KT], dv[:, 2:KT])
                w1sb[p] = wt

            def load_w(p):
                load_w1(p)
                w2sb[p] = wpool.tile([128, KT * INNER], BF16, tag=f"w2{p}",
                                     name=f"w2{p}sb")
                nc.sync.dma_start(
                    w2sb[p][:].rearrange("p (kt i) -> p kt i", kt=KT),
                    w2[p].rearrange("(kt p) i -> p kt i", p=128))

            for s, feat in (("a", fa8), ("b", fb8)):
                fts = []
                for kt in range(KT):
                    if s == "a" and kt == 0:
                        load_w1("q")
                    ft = fpool.tile([64, 2 * BN], F8E4, tag=f"f{kt}")
                    nc.sync.dma_start(ft[:], feat[64 * kt:64 * (kt + 1), :])
                    fts.append(ft)
                if s == "a":
                    w2sb["q"] = wpool.tile([128, KT * INNER], BF16, tag="w2q",
                                           name="w2qsb")
                    nc.sync.dma_start(
                        w2sb["q"][:].rearrange("p (kt i) -> p kt i", kt=KT),
                        w2["q"].rearrange("(kt p) i -> p kt i", p=128))
                    load_w("k")
                    load_w("v")
                for p in "qkv":
                    hts = []
                    for t in range(CT):
                        psH = psHp.tile([128, 1024], F32, tag="psH")
                        for lo, hi in CH:
                            for kt in range(KT):
                                nc.tensor.matmul(
                                    psH[:, lo:hi],
                                    w1sb[p][:].rearrange(
                                        "p (kt two c) -> p kt two c",
                                        kt=KT, two=2)[
                                        :, kt, :, 128 * t:128 * t + 128],
                                    fts[kt][:].rearrange(
                                        "p (two n) -> p two n", two=2)[
                                        :, :, lo:hi],
                                    start=(kt == 0), stop=(kt == KT - 1),
                                    perf_mode=mybir.MatmulPerfMode.DoubleRow)
                        ht = hpool.tile([128, BN], BF16, tag=f"h{t}")
                        if t % 2 == 0:
                            nc.scalar.activation(ht[:], psH[:, 0:BN], AF.Relu)
                        else:
                            nc.vector.tensor_scalar_max(ht[:], psH[:, 0:BN],
                                                        0.0)
                        hts.append(ht)
                    psOs = [psOp.tile([INNER, 512], F32, tag="psOa",
                                      name="psOa"),
                            psOp.tile([INNER, 512], F32, tag="psOb",
                                      name="psOb")]
                    ot = opool.tile([INNER, BN], F16, tag="out")
                    for ci, (lo, hi) in enumerate(CH):
                        for kt in range(KT):
                            nc.tensor.matmul(
                                psOs[ci][:, 0:hi - lo],
                                w2sb[p][:, INNER * kt:INNER * (kt + 1)],
                                hts[kt][:, lo:hi],
                                start=(kt == 0), stop=(kt == KT - 1))
                        nc.scalar.copy(ot[:, lo:hi], psOs[ci][:, 0:hi - lo])
                        nc.sync.dma_start(outs[(s, p)][:, lo:hi],
                                          ot[:, lo:hi])

    _split_multi_waits(nc)
    return nc


# ---------------------------------------------------------------- prog2

def build_prog2():
    """Attention program, sharded over p (this core's 8 b-batches).

    Inputs (f32r unless noted):
      kaT, qaT      [INNER, B*N]   a-side K^T / Q^T, i on partitions
      qbT, kbT      [INNER, BN]    this core's b-side chunks
      vaL, vaR      [N, 32*128]    per q-pair j: [va[2j] | 0], [0 | va[2j+1]]
      vbL, vbR      [N, 8*128]     per p: [vb[p] | 0], [0 | vb[p]]
      vhat_bT2 f32  [128, BN]      v̂b^T twice (rows 0:64 and 64:128)
      vhat_aT2 f32  [128, B*N//2]  v̂a^T in chunk-pair layout
      master1, master8 [128, 320]  reduce-selector constants
    Outputs (f32):
      out1 [64, PB]   path1 per-(q, p) sums over n of cos1
      out2 [128, 4]   path2 sums; row r: chunk=r//8, p=r%8; q=4*(r//8)+col
    """
    nc = bass.Bass("TRN2", target_bir_lowering=False, debug=False,
                   num_devices=CORES)
    din = {}
    for name, shape, dt in [
        ("kaTdr", [32, 2 * B * MP], F8E4), ("qaTdr", [32, 2 * B * N], F8E4),
        ("qbTdr", [32, 2 * BN], F8E4), ("kbTdr", [32, 2 * PB * MP], F8E4),
        ("vaL", [N, (B // 2) * 128], F16), ("vaR", [N, (B // 2) * 128], F16),
        ("vbL", [N, PB * 128], F16), ("vbR", [N, PB * 128], F16),
    ]:
        din[name] = nc.dram_tensor(name, shape, dt, kind="ExternalInput").ap()
    as1o = nc.dram_tensor("as1o", [128, 32 * BN], BF16,
                          kind="ExternalOutput").ap()
    as2o = nc.dram_tensor("as2o", [128, 32 * 800], BF16,
                          kind="ExternalOutput").ap()

    CH1 = [(0, 512), (512, BN)]          # path1 column chunks of (p n)
    E2W = 1536                           # path2 score-chunk width (3 banks)
    NQ2 = (B * N) // E2W                 # 4 full chunks + 256 remainder
    E2CH = [(E2W * j, min(E2W * (j + 1), B * N)) for j in range(NQ2 + 1)]

    with tile.TileContext(nc) as tc:
        from contextlib import ExitStack
        with ExitStack() as ctx:
            inp = ctx.enter_context(tc.tile_pool(name="inp", bufs=1))
            sb = {}

            def load(name, ap=None, cols=None, cname=None):
                ap = din[name] if ap is None else ap
                if cols is not None:
                    ap = ap[:, cols[0]:cols[1]]
                cname = cname or name
                t = inp.tile(list(ap.shape), ap.dtype, tag=cname,
                             name=f"sb_{cname}")
                nc.sync.dma_start(t[:], ap[:])
                sb[cname] = t

            # path1-critical tensors first (chunked so compute starts early)
            # kaTdr in 4 q-chunks of 16 (both DR halves per chunk)
            ka_t = inp.tile([32, 2 * B * MP], F8E4, tag="kaTdr",
                            name="sb_kaTdr")
            sb["kaTdr"] = ka_t
            ka3d = din["kaTdr"].rearrange("p (two q m) -> p two q m",
                                          two=2, q=B)
            ka3s = ka_t[:].rearrange("p (two q m) -> p two q m", two=2, q=B)
            nc.sync.dma_start(ka3s[:, :, 0:16, :], ka3d[:, :, 0:16, :])
            load("qbTdr")
            load("vaL", cols=(0, 512), cname="vaL0")
            load("vaR", cols=(0, 512), cname="vaR0")
            for c in range(1, 4):
                nc.sync.dma_start(ka3s[:, :, 16 * c:16 * (c + 1), :],
                                  ka3d[:, :, 16 * c:16 * (c + 1), :])
            # vaL/vaR tail in 2 big chunks each, sliced into per-512 views
            for nm in ("vaL", "vaR"):
                big = inp.tile([N, 3584], F16, tag=f"{nm}big",
                               name=f"sb_{nm}big")
                nc.sync.dma_start(big[:, 0:1536], din[nm][:, 512:2048])
                nc.sync.dma_start(big[:, 1536:3584], din[nm][:, 2048:4096])
                for c in range(1, 8):
                    sb[f"{nm}{c}"] = None
                sb[f"{nm}big"] = big
            # qaTdr in 2 n-chunks (both DR halves per chunk)
            qa_t = inp.tile([32, 2 * B * N], F8E4, tag="qaTdr",
                            name="sb_qaTdr")
            sb["qaTdr"] = qa_t
            qa3d = din["qaTdr"].rearrange("p (two n) -> p two n", two=2)
            qa3s = qa_t[:].rearrange("p (two n) -> p two n", two=2)
            nc.sync.dma_start(qa3s[:, :, 0:3200], qa3d[:, :, 0:3200])
            nc.sync.dma_start(qa3s[:, :, 3200:6400], qa3d[:, :, 3200:6400])
            for name in ("kbTdr", "vbL", "vbR"):
                load(name)

            epool = ctx.enter_context(tc.tile_pool(name="epool", bufs=E1_BUFS))
            mpool = ctx.enter_context(tc.tile_pool(name="mpool", bufs=M_BUFS))
            fin = ctx.enter_context(tc.tile_pool(name="fin", bufs=1))

            # ---------------- path 1: per q-pair over this core's (p n) ----
            with (
                tc.tile_pool(name="ps_s1", bufs=S1_BUFS, space="PSUM") as ps_s1,
                tc.tile_pool(name="ps_a1", bufs=int(os.environ.get("K_A1_BUFS","2")), space="PSUM") as ps_a1,
            ):
                ka3 = sb["kaTdr"][:].rearrange("p (two q m) -> p two q m",
                                               two=2, q=B)
                qb3 = sb["qbTdr"][:].rearrange("p (two n) -> p two n", two=2)
                for j in range(B // 2):
                    q0, q1 = 2 * j, 2 * j + 1
                    Ss = [ps_s1.tile([100, 1024], F32, tag="S1", name="S1a"),
                          ps_s1.tile([100, 1024], F32, tag="S1", name="S1b")]
                    for lo, hi in CH1:
                        nc.tensor.matmul(
                            Ss[0][:, lo:hi], ka3[:, :, q0, 0:N],
                            qb3[:, :, lo:hi], start=True, stop=True,
                            perf_mode=DR)
                        nc.tensor.matmul(
                            Ss[1][:, lo:hi], ka3[:, :, q1, 0:N],
                            qb3[:, :, lo:hi], start=True, stop=True,
                            perf_mode=DR)
                    Es = []
                    for S in Ss:
                        E = epool.tile([100, BN], F16, tag="E1")
                        nc.scalar.activation(E[:], S[:, 0:BN], AF.Exp,
                                             scale=0.125)
                        Es.append(E)
                    As = mpool.tile([128, BN], BF16, tag="As1")
                    for ci, (lo, hi) in enumerate(CH1):
                        Ac = (ps_a1.tile([128, 512], F32, tag="A1a", name="Aa")
                              if ci == 0 else
                              ps_a1.tile([128, 512], F32, tag="A1b", name="Ab"))
                        vaLs = (sb["vaL0"][:, 128 * j:128 * (j + 1)]
                                if j < 4 else
                                sb["vaLbig"][:, 128 * j - 512:
                                             128 * (j + 1) - 512])
                        vaRs = (sb["vaR0"][:, 128 * j:128 * (j + 1)]
                                if j < 4 else
                                sb["vaRbig"][:, 128 * j - 512:
                                             128 * (j + 1) - 512])
                        nc.tensor.matmul(Ac[:, 0:hi - lo], vaLs,
                                         Es[0][:, lo:hi],
                                         start=True, stop=False)
                        nc.tensor.matmul(Ac[:, 0:hi - lo], vaRs,
                                         Es[1][:, lo:hi],
                                         start=False, stop=True)
                        nc.vector.tensor_copy(As[:, lo:hi], Ac[:, 0:hi - lo])
                    nc.sync.dma_start(as1o[:, BN * j:BN * (j + 1)], As[:])



            # ---------------- path 2: per p over all (q n) -----------------
            with (
                tc.tile_pool(name="ps_s2", bufs=2, space="PSUM") as ps_s2,
                tc.tile_pool(name="ps_a2", bufs=int(os.environ.get("K_A2_BUFS", "2")), space="PSUM") as ps_a2,
            ):
                kb3 = sb["kbTdr"][:].rearrange("p (two b m) -> p two b m",
                                               two=2, b=PB)
                qa3 = sb["qaTdr"][:].rearrange("p (two n) -> p two n", two=2)
                for p in range(PB):
                    E2 = epool.tile([100, B * N], F16, tag="E2", bufs=E2_BUFS)
                    for lo, hi in E2CH:
                        S2 = ps_s2.tile([100, E2W], F32, tag="S2")
                        for l2 in range(lo, hi, 512):
                            h2 = min(l2 + 512, hi)
                            nc.tensor.matmul(
                                S2[:, l2 - lo:h2 - lo],
                                kb3[:, :, p, 0:N],
                                qa3[:, :, l2:h2],
                                start=True, stop=True, perf_mode=DR)
                        nc.scalar.activation(E2[:, lo:hi], S2[:, 0:hi - lo],
                                             AF.Exp, scale=0.125)
                    for g in range(4):  # groups of two chunk-pairs (1600 cols)
                        As2 = mpool.tile([128, 800], BF16, tag="As2", bufs=M2_BUFS)
                        for h in range(2):
                            j2 = 2 * g + h
                            c0 = 800 * j2
                            A2 = ps_a2.tile([128, 400], F32, tag="A2",
                                            name="A2")
                            nc.tensor.matmul(
                                A2[:], sb["vbL"][:, 128 * p:128 * (p + 1)],
                                E2[:, c0:c0 + 400], start=True, stop=False)
                            nc.tensor.matmul(
                                A2[:], sb["vbR"][:, 128 * p:128 * (p + 1)],
                                E2[:, c0 + 400:c0 + 800],
                                start=False, stop=True)
                            nc.vector.tensor_copy(As2[:, 400 * h:400 * (h + 1)],
                                                  A2[:])
                        nc.sync.dma_start(
                            as2o[:, 3200 * p + 800 * g:
                                 3200 * p + 800 * (g + 1)], As2[:])

    _split_multi_waits(nc)
    return nc


# ---------------------------------------------------------------- host

_progs = {}


def _install_compile_cache():
    """Persist compiled NEFF-wrapped custom calls across processes: walrus
    compilation takes tens of seconds per program and bass2jax recompiles
    in every fresh process otherwise."""
    import hashlib
    import pathlib
    from concourse import bass2jax
    if getattr(bass2jax, "_ant_disk_cache", False):
        return
    bass2jax._ant_disk_cache = True
    orig = bass2jax.neuronx_cc_hook
    cdir = pathlib.Path(os.environ.get("BASS_NEFF_CACHE",
                                       "/tmp/bass_neff_cache"))
    try:
        cdir.mkdir(parents=True, exist_ok=True)
    except OSError:
        return

    def cached_hook(code, code_format, platform_version, file_prefix):
        try:
            key = hashlib.sha256(
                bytes(code) + b"|" + bytes(code_format)).hexdigest()
            path = cdir / f"{key}.neffcall"
            if path.exists():
                return 0, path.read_bytes()
        except Exception:
            return orig(code, code_format, platform_version, file_prefix)
        rc, blob = orig(code, code_format, platform_version, file_prefix)
        if rc == 0:
            try:
                tmp = path.with_suffix(f".tmp{os.getpid()}")
                tmp.write_bytes(blob)
                tmp.rename(path)
            except OSError:
                pass
        return rc, blob

    bass2jax.neuronx_cc_hook = cached_hook
    try:
        import libneuronxla
        if libneuronxla.neuronx_cc is orig:
            libneuronxla.neuronx_cc = cached_hook
    except ImportError:
        pass


def _get_progs():
    if "p1" not in _progs:
        _install_compile_cache()
        _progs["p1"] = build_prog1()
        _progs["p2"] = build_prog2()
    return _progs["p1"], _progs["p2"]


def _masters():
    import ml_dtypes
    m1 = np.zeros((128, 320), ml_dtypes.bfloat16)
    m1[0:64, 128] = 1.0   # up-plane (rows 0:64 of rhs) -> out row q
    m1[64:128, 129] = 1.0  # down-plane -> out row q+1
    m8 = np.zeros((128, 320), ml_dtypes.bfloat16)
    m8[0:64, 128] = 1.0
    m8[64:128, 136] = 1.0  # down-plane -> out row r0+8
    return m1, m8


def _dr_pack_k(x, pad_to=None):
    """Pack [K, M] (K contraction, even) into DoubleRow layout
    [K//2, 2*M] fp8e4 with k = (K//2)*s + p."""
    import ml_dtypes
    K = x.shape[0]
    h = K // 2
    arr = x.reshape(2, h, *x.shape[1:]).transpose(1, 0, *range(2, x.ndim + 1))
    return np.ascontiguousarray(arr.reshape(h, -1).astype(
        ml_dtypes.float8_e4m3fn))


def _dr_pack_k_padded(x, nblk, blk, pad):
    """[K, nblk*blk] -> DR fp8 [K//2, 2*nblk*pad] with each blk padded."""
    import ml_dtypes
    K = x.shape[0]
    h = K // 2
    a = x.reshape(2, h, nblk, blk).transpose(1, 0, 2, 3)
    z = np.zeros((h, 2, nblk, pad), np.float32)
    z[:, :, :, 0:blk] = a
    return np.ascontiguousarray(z.reshape(h, -1).astype(
        ml_dtypes.float8_e4m3fn))


def kernel(features_a, features_b, Wq1, Wq2, Wk1, Wk2, Wv1, Wv2):
    import ml_dtypes
    nc1, nc2 = _get_progs()
    cc = np.ascontiguousarray

    fa = np.asarray(features_a, np.float32).reshape(B, C, N)
    fb = np.asarray(features_b, np.float32).reshape(B, C, N)

    def feat_dr(f_core):  # [PB, C, N] -> [KT1*64, 2*BN] fp8 DR
        fT = f_core.transpose(1, 0, 2).reshape(C, BN)
        a = fT.reshape(KT1, 2, 64, BN).transpose(0, 2, 1, 3)
        return cc(a.reshape(KT1 * 64, 2 * BN).astype(ml_dtypes.float8_e4m3fn))

    def w1_dr(W):  # [C, C] -> [KT1*64, 2*C] fp8 DR
        a = np.asarray(W, np.float32).reshape(KT1, 2, 64, C).transpose(
            0, 2, 1, 3)
        return cc(a.reshape(KT1 * 64, 2 * C).astype(ml_dtypes.float8_e4m3fn))

    ws = {"wq1dr": w1_dr(Wq1), "wk1dr": w1_dr(Wk1), "wv1dr": w1_dr(Wv1)}
    ws.update({k: cc(np.asarray(v, np.float32).astype(ml_dtypes.bfloat16))
               for k, v in (("wq2", Wq2), ("wk2", Wk2), ("wv2", Wv2))})

    in1 = [dict(fa8dr=feat_dr(fa[PB * i:PB * (i + 1)]),
                fb8dr=feat_dr(fb[PB * i:PB * (i + 1)]), **ws)
           for i in range(CORES)]
    res1 = run_bass_kernel_spmd(nc1, in1, core_ids=list(range(CORES)))

    def gather(name):
        return np.concatenate([res1.results[i][name] for i in range(CORES)],
                              axis=1)

    qaT, kaT, vaT = gather("qaT8"), gather("kaT8"), gather("vaT8")
    qbT = [res1.results[i]["qbT8"] for i in range(CORES)]
    kbT = [res1.results[i]["kbT8"] for i in range(CORES)]
    vbT = [res1.results[i]["vbT8"] for i in range(CORES)]

    # a-side derived tensors (shared by all cores)
    vaT32 = vaT.astype(np.float32)
    va_nm = cc(vaT.T)                       # [B*N, INNER] fp16
    na = np.maximum(np.sqrt((vaT32 * vaT32).sum(0)), EPS)
    vhat_aT = vaT32 / na[None, :]
    vaL = np.zeros((N, (B // 2) * 128), np.float16)
    vaR = np.zeros((N, (B // 2) * 128), np.float16)
    for j in range(B // 2):
        vaL[:, 128 * j:128 * j + 64] = va_nm[N * 2 * j:N * (2 * j + 1)]
        vaR[:, 128 * j + 64:128 * (j + 1)] = va_nm[N * (2 * j + 1):
                                                   N * (2 * j + 2)]
    vhat_aT2 = np.zeros((128, B * N // 2), np.float32)
    for j2 in range(8):
        vhat_aT2[0:64, 400 * j2:400 * (j2 + 1)] = \
            vhat_aT[:, 800 * j2:800 * j2 + 400]
        vhat_aT2[64:128, 400 * j2:400 * (j2 + 1)] = \
            vhat_aT[:, 800 * j2 + 400:800 * (j2 + 1)]
    m1, m8 = _masters()

    kaTdr = _dr_pack_k_padded(kaT.astype(np.float32), B, N, MP)
    qaTdr = _dr_pack_k(qaT.astype(np.float32))
    in2 = []
    vhat_bTs = []
    for i in range(CORES):
        vbT32 = vbT[i].astype(np.float32)
        vb_nm = cc(vbT[i].T)                # [BN, INNER] fp16
        nb = np.maximum(np.sqrt((vbT32 * vbT32).sum(0)), EPS)
        vhat_bT = vbT32 / nb[None, :]
        vbL = np.zeros((N, PB * 128), np.float16)
        vbR = np.zeros((N, PB * 128), np.float16)
        for p in range(PB):
            vbL[:, 128 * p:128 * p + 64] = vb_nm[N * p:N * (p + 1)]
            vbR[:, 128 * p + 64:128 * (p + 1)] = vb_nm[N * p:N * (p + 1)]
        vhat_bTs.append(vhat_bT)
        in2.append(dict(
            kaTdr=kaTdr, qaTdr=qaTdr,
            qbTdr=_dr_pack_k(qbT[i].astype(np.float32)),
            kbTdr=_dr_pack_k_padded(kbT[i].astype(np.float32), PB, N, MP),
            vaL=vaL, vaR=vaR, vbL=vbL, vbR=vbR))
    res2 = run_bass_kernel_spmd(nc2, in2, core_ids=list(range(CORES)))

    sim = np.zeros((B, B), np.float32)
    for i in range(CORES):
        r = res2.results[i]
        # path1: As1 col-block 800j = pair j (rows 0:64 -> q=2j,
        # rows 64:128 -> q=2j+1, cols (p, n)); dot/ny2 on host
        as1 = np.asarray(r["as1o"], np.float32).reshape(128, 32, 800)
        vb_h = vhat_bTs[i]                              # [64 i, 800 (p n)]
        ny2_1 = np.empty((64, 800), np.float32)
        dot1 = np.empty((64, 800), np.float32)
        ny2_1[0::2] = (as1[0:64] ** 2).sum(0)
        ny2_1[1::2] = (as1[64:128] ** 2).sum(0)
        dot1[0::2] = np.einsum('ijc,ic->jc', as1[0:64], vb_h)
        dot1[1::2] = np.einsum('ijc,ic->jc', as1[64:128], vb_h)
        cos1 = dot1 / np.maximum(np.sqrt(ny2_1), EPS)
        sim1 = cos1.reshape(64, PB, N).sum(-1)          # [q, p]

        # path2: As2 cols 3200p + 800g + 400h + c; rows 0:64 ->
        # qn = 800*(2g+h)+c, rows 64:128 -> +400; vhat_a [64, (g,h,half,c)]
        as2 = np.asarray(r["as2o"], np.float32).reshape(128, PB, 4, 2, 400)
        va4 = vhat_aT.reshape(64, 4, 2, 2, 400)         # [i, g, h, half, c]
        ny_lo = (as2[0:64] ** 2).sum(0).reshape(PB, 8, 400)
        ny_hi = (as2[64:128] ** 2).sum(0).reshape(PB, 8, 400)
        ny2_2 = np.concatenate([ny_lo, ny_hi], axis=2).reshape(PB, B * N)
        d_lo = np.einsum('ipghc,ighc->pghc', as2[0:64], va4[:, :, :, 0])
        d_hi = np.einsum('ipghc,ighc->pghc', as2[64:128], va4[:, :, :, 1])
        dot2 = np.concatenate([d_lo.reshape(PB, 8, 400),
                               d_hi.reshape(PB, 8, 400)],
                              axis=2).reshape(PB, B * N)
        cos2 = dot2 / np.maximum(np.sqrt(ny2_2), EPS)
        sim2 = cos2.reshape(PB, B, N).sum(-1)           # [p, q]

        sim[PB * i:PB * (i + 1)] = (sim1.T + sim2) / N
    return sim

